# revision 21
# baseline (speedup 1.0000x reference)
"""Multi-head attention Bass/Tile kernel for Trainium2, SPMD over 8 NeuronCores.

Problem: B=16, S=1024, D=512, H=8, dk=64.  out = MHA(x); returns
(out [B,S,D], attn_weights [B,H,S,S]).  Data-parallel over B: each of the 8
cores processes 2 batches end-to-end (no collectives needed).

Layout strategy per core (BL=2 local batches):
  xT   [din=128p, 4t, S]      <- PE-transpose of x_b
  QT/KT[dout=128p, 4t, S]     = W.T @ x.T   (head h lives at partitions
                                 64*(h%2) of tile h//2)
  V    [tok=128p, 8t, D]      = x @ Wv  (natural layout; lhsT for ctx matmul)
  per (b,h), per q-tile(128):
    scores psum [128q, 1024k] = QT_h.T @ KT_h   (K=dk=64)
    E = exp(scores/8)  (ACT, accum_out -> row sums)
    attn = E * recip(sums)   (DVE per-partition scalar) -> DMA out
    attnT [k=128p, 8t, 512q] <- PE-transpose of attn tiles
  per q-chunk(512): ctxT[dk,q] psum += V_h.T @ attnT   (K=k tiles)
  out[q, D] = sum_t ctxT_t.T @ Wo_t + bo  -> DMA out
"""

import os
import sys
import tempfile

import numpy as np

sys.path.insert(0, "/opt/trn_rl_repo")

B, S, D_MODEL, N_HEADS = 16, 1024, 512, 8
D_K = D_MODEL // N_HEADS        # 64
N_CORES = 8
BL = B // N_CORES               # 2 local batches per core
P = 128                         # partitions
DIN_T = D_MODEL // P            # 4 din tiles
QT_T = S // P                   # 8 q tiles per batch
KC = S // 512                   # 2 k chunks of 512
SCALE = 1.0 / float(np.sqrt(D_K))

# compute dtype for matmul operands: "float32" (exact, 4 cyc/row) or
# "float32r" (tf32-like, 1 cyc/row at free>=256)
MM_DTYPE = os.environ.get("MHA_MM_DTYPE", "float32r")


def _build(nc_holder=[]):
    import concourse.bass as bass
    import concourse.tile as tile
    from concourse import bacc, mybir
    from concourse.masks import make_identity

    fp32 = mybir.dt.float32
    cdt = getattr(mybir.dt, MM_DTYPE)
    AF = mybir.ActivationFunctionType

    nc = bacc.Bacc("TRN2", target_bir_lowering=False, debug=False,
                   num_devices=N_CORES)

    x = nc.dram_tensor("x", [BL, S, D_MODEL], fp32, kind="ExternalInput").ap()
    Wq = nc.dram_tensor("Wq", [D_MODEL, D_MODEL], fp32, kind="ExternalInput").ap()
    Wk = nc.dram_tensor("Wk", [D_MODEL, D_MODEL], fp32, kind="ExternalInput").ap()
    Wv = nc.dram_tensor("Wv", [D_MODEL, D_MODEL], fp32, kind="ExternalInput").ap()
    Wo = nc.dram_tensor("Wo", [D_MODEL, D_MODEL], fp32, kind="ExternalInput").ap()
    bq = nc.dram_tensor("bq", [D_MODEL], fp32, kind="ExternalInput").ap()
    bk = nc.dram_tensor("bk", [D_MODEL], fp32, kind="ExternalInput").ap()
    bv = nc.dram_tensor("bv", [D_MODEL], fp32, kind="ExternalInput").ap()
    bo = nc.dram_tensor("bo", [D_MODEL], fp32, kind="ExternalInput").ap()
    out = nc.dram_tensor("out", [BL, S, D_MODEL], fp32, kind="ExternalOutput").ap()
    attn = nc.dram_tensor("attn", [BL, N_HEADS, S, S], fp32,
                          kind="ExternalOutput").ap()

    from contextlib import ExitStack

    with tile.TileContext(nc) as tc, ExitStack() as ctx:
        const = ctx.enter_context(tc.tile_pool(name="const", bufs=1))
        bigp = ctx.enter_context(tc.tile_pool(name="bigp", bufs=1))   # per-batch 2MB tiles
        midp = ctx.enter_context(tc.tile_pool(name="midp", bufs=2))   # attnT ring
        rowp = ctx.enter_context(tc.tile_pool(name="rowp", bufs=3))   # E/attn rows
        smal = ctx.enter_context(tc.tile_pool(name="smal", bufs=4))   # stats
        outp = ctx.enter_context(tc.tile_pool(name="outp", bufs=2))
        # psum pools
        ps_mm = ctx.enter_context(tc.tile_pool(name="ps_mm", bufs=4, space="PSUM"))

        ident_f32 = const.tile([P, P], fp32)
        make_identity(nc, ident_f32)
        ident = const.tile([P, P], cdt)
        nc.vector.tensor_copy(ident, ident_f32)

        ones_f32 = const.tile([1, P], fp32)
        nc.vector.memset(ones_f32, 1.0)
        ones = const.tile([1, P], cdt)
        nc.vector.tensor_copy(ones, ones_f32)
        ones_c64 = const.tile([P, 64], fp32)
        nc.vector.memset(ones_c64, 1.0)

        # weights in [din_part, din_tile, dout] layout, rounded to compute dtype
        def load_w(w_dram, name):
            raw = bigp.tile([P, QT_T, D_MODEL], fp32, tag="x_sb",
                            name=f"{name}_raw")[:, 0:DIN_T, :]
            nc.sync.dma_start(out=raw, in_=w_dram.rearrange("(t p) n -> p t n", p=P))
            r = const.tile([P, DIN_T, D_MODEL], cdt, tag=f"{name}r")
            nc.vector.tensor_copy(r, raw)
            return r

        Wq_sb = load_w(Wq, "wq")
        Wk_sb = load_w(Wk, "wk")
        Wv_sb = load_w(Wv, "wv")
        Wo_sb = load_w(Wo, "wo")

        # per-partition bias layout [128, 4] for Q/K (bias on dout partition dim)
        def load_b_part(b_dram, name):
            t = const.tile([P, DIN_T], fp32, tag=f"{name}p")
            nc.sync.dma_start(out=t, in_=b_dram.rearrange("(t p) -> p t", p=P))
            return t

        bq_sb = load_b_part(bq, "bq")
        bk_sb = load_b_part(bk, "bk")

        # row bias layout [1, 512] for V / out (bias on dout free dim)
        def load_b_row(b_dram, name):
            raw = const.tile([1, D_MODEL], fp32, tag=f"{name}rraw")
            nc.sync.dma_start(out=raw, in_=b_dram[None, :])
            if cdt == fp32:
                return raw
            r = const.tile([1, D_MODEL], cdt, tag=f"{name}rr")
            nc.vector.tensor_copy(r, raw)
            return r

        bv_sb = load_b_row(bv, "bv")
        bo_sb = load_b_row(bo, "bo")

        for b in range(BL):
            # ---- load x_b and build xT (rounded at DVE evac) ----
            x_sb = bigp.tile([P, QT_T, D_MODEL], fp32, tag="x_sb")
            nc.sync.dma_start(out=x_sb,
                              in_=x[b].rearrange("(t p) d -> p t d", p=P))
            xT = bigp.tile([P, DIN_T, S], cdt, tag="xT")
            for dt_i in range(DIN_T):
                for qg in range(2):             # groups of 4 q-tiles
                    pst = ps_mm.tile([P, 1024], fp32, tag="big", name="pst_x")[:, 0:512]
                    for j in range(4):
                        qt = qg * 4 + j
                        nc.tensor.transpose(
                            pst[:, j * P:(j + 1) * P],
                            x_sb[:, qt, dt_i * P:(dt_i + 1) * P],
                            ident_f32)
                    nc.vector.tensor_copy(
                        xT[:, dt_i, qg * 512:(qg + 1) * 512], pst)

            # ---- projections ----
            QT = bigp.tile([P, DIN_T, S], cdt, tag="QT")
            KT = bigp.tile([P, DIN_T, S], cdt, tag="KT")
            for W_sb, b_sb, dst in ((Wq_sb, bq_sb, QT), (Wk_sb, bk_sb, KT)):
                for ot in range(DIN_T):
                    for qc in range(2):
                        ps = ps_mm.tile([P, 1024], fp32, tag="big", name=f"ps_p{ot}_{qc}")[:, 0:512]
                        for kt in range(DIN_T):
                            nc.tensor.matmul(
                                ps,
                                W_sb[:, kt, ot * P:(ot + 1) * P],
                                xT[:, kt, qc * 512:(qc + 1) * 512],
                                start=(kt == 0), stop=(kt == DIN_T - 1))
                        nc.vector.tensor_scalar_add(
                            dst[:, ot, qc * 512:(qc + 1) * 512], ps,
                            b_sb[:, ot:ot + 1])

            # V with a ones column per head: [tok, 8h, 65] (col 64 = 1.0 so
            # the ctx matmul's row 64 accumulates the softmax denominator)
            V_aug = bigp.tile([P, QT_T, N_HEADS * 65], cdt, tag="V_aug")
            nc.vector.tensor_copy(
                V_aug.rearrange("p t (h c) -> p t h c", c=65)[:, :, :, 64:65],
                ones_c64.rearrange("p (t h) -> p t h", h=N_HEADS)[:, :, :, None])
            for tt in range(QT_T):
                ps = ps_mm.tile([P, 1024], fp32, tag="big", name=f"ps_v{tt}")[:, 0:512]
                for kt in range(DIN_T):
                    nc.tensor.matmul(
                        ps,
                        xT[:, kt, tt * P:(tt + 1) * P],
                        Wv_sb[:, kt, :],
                        start=(kt == 0), stop=False)
                nc.tensor.matmul(ps, ones, bv_sb, start=False, stop=True)
                nc.vector.tensor_copy(
                    V_aug[:, tt, :].rearrange("p (h c) -> p h c", c=65)[:, :, 0:64],
                    ps.rearrange("p (h c) -> p h c", c=64))

            ctxT = bigp.tile([P, DIN_T, S], cdt, tag="ctxT")

            # ---- attention per head ----
            for h in range(N_HEADS):
                hp = 64 * (h % 2)
                ht = h // 2
                QT_h = QT[hp:hp + 64, ht, :]
                KT_h = KT[hp:hp + 64, ht, :]
                for qc in range(2):               # q chunks of 512
                    # unnormalized expT [k, 8kt, 512q] straight from matmul+exp
                    ET = midp.tile([P, QT_T, 512], cdt, tag="ET")
                    for kp in range(QT_T // 2):
                        ps_t = ps_mm.tile([P, 1024], fp32, tag="big",
                                          name=f"ps_t{kp}")
                        for i in range(2):
                            kt = kp * 2 + i
                            nc.tensor.matmul(
                                ps_t[:, i * 512:(i + 1) * 512],
                                KT_h[:, kt * P:(kt + 1) * P],
                                QT_h[:, qc * 512:(qc + 1) * 512],
                                start=True, stop=True)
                        nc.scalar.activation(
                            ET[:, kp * 2:kp * 2 + 2, :],
                            ps_t.rearrange("p (t q) -> p t q", q=512),
                            AF.Exp, scale=SCALE)
                    # normalized attn rows [q, k] for the output
                    sums4 = smal.tile([P, 4], fp32, tag="sums4")
                    recip4 = smal.tile([P, 4], fp32, tag="recip4")
                    Es = []
                    for j in range(4):            # q tiles of 128
                        qt = qc * 4 + j
                        ps_s = ps_mm.tile([P, 1024], fp32, tag="big",
                                          name=f"ps_s_{b}_{h}_{qt}")
                        for kc in range(KC):
                            nc.tensor.matmul(
                                ps_s[:, kc * 512:(kc + 1) * 512],
                                QT_h[:, qt * P:(qt + 1) * P],
                                KT_h[:, kc * 512:(kc + 1) * 512],
                                start=True, stop=True)
                        E = rowp.tile([P, S], fp32, tag="E", bufs=5,
                                      name=f"E_{b}_{h}_{qt}")
                        nc.scalar.activation(
                            E, ps_s, AF.Exp, scale=SCALE,
                            accum_out=sums4[:, j:j + 1])
                        Es.append(E)
                    nc.vector.reciprocal(recip4, sums4)
                    for j in range(4):
                        qt = qc * 4 + j
                        attn_sb = rowp.tile([P, S], fp32, tag="attn_sb", bufs=2,
                                            name=f"attn_sb_{b}_{h}_{qt}")
                        nc.vector.tensor_scalar_mul(attn_sb, Es[j],
                                                    recip4[:, j:j + 1])
                        nc.sync.dma_start(
                            out=attn[b, h, qt * P:(qt + 1) * P, :],
                            in_=attn_sb)
                    # ctx for this q-chunk, accumulate over k tiles; row 64
                    # picks up the denominator via V_aug's ones column
                    psc = ps_mm.tile([P, 1024], fp32, tag="big",
                                     name=f"psc_{b}_{h}_{qc}")[:, 0:512]
                    for kt in range(QT_T):
                        nc.tensor.matmul(
                            psc[0:65, :],
                            V_aug[:, kt, h * 65:h * 65 + 65],
                            ET[:, kt, :],
                            start=(kt == 0), stop=(kt == QT_T - 1))
                    # broadcast 1/denominator across the 64 dk partitions
                    sums_b = rowp.tile([64, 512], fp32, tag="sums_b", bufs=2)
                    nc.vector.tensor_copy(sums_b[0:1, :], psc[64:65, :])
                    nc.gpsimd.partition_broadcast(sums_b, sums_b[0:1, :])
                    rb_sb = sums_b
                    nc.vector.reciprocal(rb_sb, sums_b)
                    nc.vector.tensor_mul(
                        ctxT[hp:hp + 64, ht, qc * 512:(qc + 1) * 512],
                        psc[0:64, :], rb_sb)

            # ---- output projection ----
            for qt in range(QT_T):
                ps = ps_mm.tile([P, 1024], fp32, tag="big", name=f"ps_o{qt}")[:, 0:512]
                for dt_i in range(DIN_T):
                    nc.tensor.matmul(
                        ps,
                        ctxT[:, dt_i, qt * P:(qt + 1) * P],
                        Wo_sb[:, dt_i, :],
                        start=(dt_i == 0), stop=False)
                nc.tensor.matmul(ps, ones, bo_sb, start=False, stop=True)
                o_sb = outp.tile([P, D_MODEL], fp32, tag="o_sb")
                nc.vector.tensor_copy(o_sb, ps)
                nc.sync.dma_start(out=out[b, qt * P:(qt + 1) * P, :], in_=o_sb)

    nc.compile()
    return nc


_CACHE = {}


def _get_nc():
    key = MM_DTYPE
    if key not in _CACHE:
        _CACHE[key] = _build()
    return _CACHE[key]


def kernel(x, Wq, bq, Wk, bk, Wv, bv, Wo, bo, trace=False, tmpdir=None):
    from concourse.bass_utils import run_bass_kernel_spmd

    nc = _get_nc()
    shared = {
        "Wq": np.ascontiguousarray(Wq, np.float32),
        "Wk": np.ascontiguousarray(Wk, np.float32),
        "Wv": np.ascontiguousarray(Wv, np.float32),
        "Wo": np.ascontiguousarray(Wo, np.float32),
        "bq": np.ascontiguousarray(bq, np.float32),
        "bk": np.ascontiguousarray(bk, np.float32),
        "bv": np.ascontiguousarray(bv, np.float32),
        "bo": np.ascontiguousarray(bo, np.float32),
    }
    x = np.ascontiguousarray(x, np.float32)
    in_maps = [dict(shared, x=x[c * BL:(c + 1) * BL]) for c in range(N_CORES)]
    res = run_bass_kernel_spmd(
        nc, in_maps, core_ids=list(range(N_CORES)), trace=trace,
        tmpdir=tmpdir)
    out = np.concatenate([res.results[c]["out"] for c in range(N_CORES)], axis=0)
    attn = np.concatenate([res.results[c]["attn"] for c in range(N_CORES)], axis=0)
    kernel.last_results = res
    return out, attn


# revision 22
# speedup vs baseline: 1.2611x; 1.2611x over previous
"""Multi-head attention Bass/Tile kernel for Trainium2, SPMD over 8 NeuronCores.

Problem: B=16, S=1024, D=512, H=8, dk=64.  out = MHA(x); returns
(out [B,S,D], attn_weights [B,H,S,S]).  Data-parallel over B: each of the 8
cores processes 2 batches end-to-end (no collectives needed).

Layout strategy per core (BL=2 local batches):
  xT   [din=128p, 4t, S]      <- PE-transpose of x_b
  QT/KT[dout=128p, 4t, S]     = W.T @ x.T   (head h lives at partitions
                                 64*(h%2) of tile h//2)
  V    [tok=128p, 8t, D]      = x @ Wv  (natural layout; lhsT for ctx matmul)
  per (b,h), per q-tile(128):
    scores psum [128q, 1024k] = QT_h.T @ KT_h   (K=dk=64)
    E = exp(scores/8)  (ACT, accum_out -> row sums)
    attn = E * recip(sums)   (DVE per-partition scalar) -> DMA out
    attnT [k=128p, 8t, 512q] <- PE-transpose of attn tiles
  per q-chunk(512): ctxT[dk,q] psum += V_h.T @ attnT   (K=k tiles)
  out[q, D] = sum_t ctxT_t.T @ Wo_t + bo  -> DMA out
"""

import os
import sys
import tempfile

import numpy as np

sys.path.insert(0, "/opt/trn_rl_repo")

B, S, D_MODEL, N_HEADS = 16, 1024, 512, 8
D_K = D_MODEL // N_HEADS        # 64
N_CORES = 8
BL = B // N_CORES               # 2 local batches per core
P = 128                         # partitions
DIN_T = D_MODEL // P            # 4 din tiles
QT_T = S // P                   # 8 q tiles per batch
KC = S // 512                   # 2 k chunks of 512
SCALE = 1.0 / float(np.sqrt(D_K))

# compute dtype for matmul operands: "float32" (exact, 4 cyc/row) or
# "float32r" (tf32-like, 1 cyc/row at free>=256)
MM_DTYPE = os.environ.get("MHA_MM_DTYPE", "float32r")


def _build(nc_holder=[]):
    import concourse.bass as bass
    import concourse.tile as tile
    from concourse import bacc, mybir
    from concourse.masks import make_identity

    fp32 = mybir.dt.float32
    cdt = getattr(mybir.dt, MM_DTYPE)
    AF = mybir.ActivationFunctionType

    nc = bacc.Bacc("TRN2", target_bir_lowering=False, debug=False,
                   num_devices=N_CORES)

    x = nc.dram_tensor("x", [BL, S, D_MODEL], fp32, kind="ExternalInput").ap()
    Wq = nc.dram_tensor("Wq", [D_MODEL, D_MODEL], fp32, kind="ExternalInput").ap()
    Wk = nc.dram_tensor("Wk", [D_MODEL, D_MODEL], fp32, kind="ExternalInput").ap()
    Wv = nc.dram_tensor("Wv", [D_MODEL, D_MODEL], fp32, kind="ExternalInput").ap()
    Wo = nc.dram_tensor("Wo", [D_MODEL, D_MODEL], fp32, kind="ExternalInput").ap()
    bq = nc.dram_tensor("bq", [D_MODEL], fp32, kind="ExternalInput").ap()
    bk = nc.dram_tensor("bk", [D_MODEL], fp32, kind="ExternalInput").ap()
    bv = nc.dram_tensor("bv", [D_MODEL], fp32, kind="ExternalInput").ap()
    bo = nc.dram_tensor("bo", [D_MODEL], fp32, kind="ExternalInput").ap()
    out = nc.dram_tensor("out", [BL, S, D_MODEL], fp32, kind="ExternalOutput").ap()
    attn = nc.dram_tensor("attn", [BL, N_HEADS, S, S], fp32,
                          kind="ExternalOutput").ap()

    from contextlib import ExitStack

    with tile.TileContext(nc) as tc, ExitStack() as ctx:
        const = ctx.enter_context(tc.tile_pool(name="const", bufs=1))
        bigp = ctx.enter_context(tc.tile_pool(name="bigp", bufs=1))   # per-batch 2MB tiles
        midp = ctx.enter_context(tc.tile_pool(name="midp", bufs=2))   # attnT ring
        rowp = ctx.enter_context(tc.tile_pool(name="rowp", bufs=3))   # E/attn rows
        smal = ctx.enter_context(tc.tile_pool(name="smal", bufs=4))   # stats
        outp = ctx.enter_context(tc.tile_pool(name="outp", bufs=2))
        # psum pools
        ps_mm = ctx.enter_context(tc.tile_pool(name="ps_mm", bufs=3, space="PSUM"))
        ps_cx = ctx.enter_context(tc.tile_pool(name="ps_cx", bufs=2, space="PSUM"))

        ident_f32 = const.tile([P, P], fp32)
        make_identity(nc, ident_f32)
        ident = const.tile([P, P], cdt)
        nc.vector.tensor_copy(ident, ident_f32)

        ones_f32 = const.tile([1, P], fp32)
        nc.vector.memset(ones_f32, 1.0)
        ones = const.tile([1, P], cdt)
        nc.vector.tensor_copy(ones, ones_f32)
        ones_c64 = const.tile([P, 64], fp32)
        nc.vector.memset(ones_c64, 1.0)

        # weights in [din_part, din_tile, dout] layout, rounded to compute dtype
        def load_w(w_dram, name):
            raw = bigp.tile([P, QT_T, D_MODEL], fp32, tag="x_sb",
                            name=f"{name}_raw")[:, 0:DIN_T, :]
            nc.sync.dma_start(out=raw, in_=w_dram.rearrange("(t p) n -> p t n", p=P))
            r = const.tile([P, DIN_T, D_MODEL], cdt, tag=f"{name}r")
            nc.vector.tensor_copy(r, raw)
            return r

        Wq_sb = load_w(Wq, "wq")
        Wk_sb = load_w(Wk, "wk")
        Wv_sb = load_w(Wv, "wv")
        Wo_sb = load_w(Wo, "wo")

        # per-partition bias layout [128, 4] for Q/K (bias on dout partition dim)
        def load_b_part(b_dram, name):
            t = const.tile([P, DIN_T], fp32, tag=f"{name}p")
            nc.sync.dma_start(out=t, in_=b_dram.rearrange("(t p) -> p t", p=P))
            return t

        bq_sb = load_b_part(bq, "bq")
        bk_sb = load_b_part(bk, "bk")

        # row bias layout [1, 512] for V / out (bias on dout free dim)
        def load_b_row(b_dram, name):
            raw = const.tile([1, D_MODEL], fp32, tag=f"{name}rraw")
            nc.sync.dma_start(out=raw, in_=b_dram[None, :])
            if cdt == fp32:
                return raw
            r = const.tile([1, D_MODEL], cdt, tag=f"{name}rr")
            nc.vector.tensor_copy(r, raw)
            return r

        bv_sb = load_b_row(bv, "bv")
        bo_sb = load_b_row(bo, "bo")

        for b in range(BL):
            # ---- load x_b and build xT (rounded at DVE evac) ----
            x_sb = bigp.tile([P, QT_T, D_MODEL], fp32, tag="x_sb")
            nc.sync.dma_start(out=x_sb,
                              in_=x[b].rearrange("(t p) d -> p t d", p=P))
            xT = bigp.tile([P, DIN_T, S], cdt, tag="xT")
            for dt_i in range(DIN_T):
                for qg in range(2):             # groups of 4 q-tiles
                    pst = ps_mm.tile([P, 1024], fp32, tag="big", name="pst_x")[:, 0:512]
                    for j in range(4):
                        qt = qg * 4 + j
                        nc.tensor.transpose(
                            pst[:, j * P:(j + 1) * P],
                            x_sb[:, qt, dt_i * P:(dt_i + 1) * P],
                            ident_f32)
                    nc.vector.tensor_copy(
                        xT[:, dt_i, qg * 512:(qg + 1) * 512], pst)

            # ---- projections ----
            QT = bigp.tile([P, DIN_T, S], cdt, tag="QT")
            KT = bigp.tile([P, DIN_T, S], cdt, tag="KT")
            for W_sb, b_sb, dst in ((Wq_sb, bq_sb, QT), (Wk_sb, bk_sb, KT)):
                for ot in range(DIN_T):
                    for qc in range(2):
                        ps = ps_mm.tile([P, 1024], fp32, tag="big", name=f"ps_p{ot}_{qc}")[:, 0:512]
                        for kt in range(DIN_T):
                            nc.tensor.matmul(
                                ps,
                                W_sb[:, kt, ot * P:(ot + 1) * P],
                                xT[:, kt, qc * 512:(qc + 1) * 512],
                                start=(kt == 0), stop=(kt == DIN_T - 1))
                        nc.vector.tensor_scalar_add(
                            dst[:, ot, qc * 512:(qc + 1) * 512], ps,
                            b_sb[:, ot:ot + 1])

            # V with a ones column per head: [tok, 8h, 65] (col 64 = 1.0 so
            # the ctx matmul's row 64 accumulates the softmax denominator)
            V_aug = bigp.tile([P, QT_T, N_HEADS * 65], cdt, tag="V_aug")
            nc.vector.tensor_copy(
                V_aug.rearrange("p t (h c) -> p t h c", c=65)[:, :, :, 64:65],
                ones_c64.rearrange("p (t h) -> p t h", h=N_HEADS)[:, :, :, None])
            for tt in range(QT_T):
                ps = ps_mm.tile([P, 1024], fp32, tag="big", name=f"ps_v{tt}")[:, 0:512]
                for kt in range(DIN_T):
                    nc.tensor.matmul(
                        ps,
                        xT[:, kt, tt * P:(tt + 1) * P],
                        Wv_sb[:, kt, :],
                        start=(kt == 0), stop=False)
                nc.tensor.matmul(ps, ones, bv_sb, start=False, stop=True)
                nc.vector.tensor_copy(
                    V_aug[:, tt, :].rearrange("p (h c) -> p h c", c=65)[:, :, 0:64],
                    ps.rearrange("p (h c) -> p h c", c=64))

            ctxT = bigp.tile([P, DIN_T, S], cdt, tag="ctxT")

            # ---- attention per head ----
            for h in range(N_HEADS):
                hp = 64 * (h % 2)
                ht = h // 2
                QT_h = QT[hp:hp + 64, ht, :]
                KT_h = KT[hp:hp + 64, ht, :]
                for qc in range(2):               # q chunks of 512
                    # unnormalized expT [k, 8kt, 512q] straight from matmul+exp
                    ET = midp.tile([P, QT_T, 512], cdt, tag="ET")
                    for kp in range(QT_T // 2):
                        ps_t = ps_mm.tile([P, 1024], fp32, tag="big",
                                          name=f"ps_t{kp}")
                        for i in range(2):
                            kt = kp * 2 + i
                            nc.tensor.matmul(
                                ps_t[:, i * 512:(i + 1) * 512],
                                KT_h[:, kt * P:(kt + 1) * P],
                                QT_h[:, qc * 512:(qc + 1) * 512],
                                start=True, stop=True)
                        nc.scalar.activation(
                            ET[:, kp * 2:kp * 2 + 2, :],
                            ps_t.rearrange("p (t q) -> p t q", q=512),
                            AF.Exp, scale=SCALE)
                    # normalized attn rows [q, k] for the output
                    sums4 = smal.tile([P, 4], fp32, tag="sums4")
                    recip4 = smal.tile([P, 4], fp32, tag="recip4")
                    Es = []
                    for j in range(4):            # q tiles of 128
                        qt = qc * 4 + j
                        ps_s = ps_mm.tile([P, 1024], fp32, tag="big",
                                          name=f"ps_s_{b}_{h}_{qt}")
                        for kc in range(KC):
                            nc.tensor.matmul(
                                ps_s[:, kc * 512:(kc + 1) * 512],
                                QT_h[:, qt * P:(qt + 1) * P],
                                KT_h[:, kc * 512:(kc + 1) * 512],
                                start=True, stop=True)
                        E = rowp.tile([P, S], fp32, tag="E", bufs=5,
                                      name=f"E_{b}_{h}_{qt}")
                        nc.scalar.activation(
                            E, ps_s, AF.Exp, scale=SCALE,
                            accum_out=sums4[:, j:j + 1])
                        Es.append(E)
                    nc.vector.reciprocal(recip4, sums4)
                    for j in range(4):
                        qt = qc * 4 + j
                        attn_sb = rowp.tile([P, S], fp32, tag="attn_sb", bufs=2,
                                            name=f"attn_sb_{b}_{h}_{qt}")
                        nc.vector.tensor_scalar_mul(attn_sb, Es[j],
                                                    recip4[:, j:j + 1])
                        nc.sync.dma_start(
                            out=attn[b, h, qt * P:(qt + 1) * P, :],
                            in_=attn_sb)
                    # ctx for this q-chunk, accumulate over k tiles; row 64
                    # picks up the denominator via V_aug's ones column
                    psc = ps_cx.tile([P, 512], fp32)
                    for kt in range(QT_T):
                        nc.tensor.matmul(
                            psc[0:65, :],
                            V_aug[:, kt, h * 65:h * 65 + 65],
                            ET[:, kt, :],
                            start=(kt == 0), stop=(kt == QT_T - 1))
                    # broadcast 1/denominator across the 64 dk partitions
                    sums_b = rowp.tile([64, 512], fp32, tag="sums_b", bufs=2)
                    nc.vector.tensor_copy(sums_b[0:1, :], psc[64:65, :])
                    nc.gpsimd.partition_broadcast(sums_b, sums_b[0:1, :])
                    rb_sb = sums_b
                    nc.vector.reciprocal(rb_sb, sums_b)
                    nc.vector.tensor_mul(
                        ctxT[hp:hp + 64, ht, qc * 512:(qc + 1) * 512],
                        psc[0:64, :], rb_sb)

            # ---- output projection ----
            for qt in range(QT_T):
                ps = ps_mm.tile([P, 1024], fp32, tag="big", name=f"ps_o{qt}")[:, 0:512]
                for dt_i in range(DIN_T):
                    nc.tensor.matmul(
                        ps,
                        ctxT[:, dt_i, qt * P:(qt + 1) * P],
                        Wo_sb[:, dt_i, :],
                        start=(dt_i == 0), stop=False)
                nc.tensor.matmul(ps, ones, bo_sb, start=False, stop=True)
                o_sb = outp.tile([P, D_MODEL], fp32, tag="o_sb")
                nc.vector.tensor_copy(o_sb, ps)
                nc.sync.dma_start(out=out[b, qt * P:(qt + 1) * P, :], in_=o_sb)

    nc.compile()
    return nc


_CACHE = {}


def _get_nc():
    key = MM_DTYPE
    if key not in _CACHE:
        _CACHE[key] = _build()
    return _CACHE[key]


def kernel(x, Wq, bq, Wk, bk, Wv, bv, Wo, bo, trace=False, tmpdir=None):
    from concourse.bass_utils import run_bass_kernel_spmd

    nc = _get_nc()
    shared = {
        "Wq": np.ascontiguousarray(Wq, np.float32),
        "Wk": np.ascontiguousarray(Wk, np.float32),
        "Wv": np.ascontiguousarray(Wv, np.float32),
        "Wo": np.ascontiguousarray(Wo, np.float32),
        "bq": np.ascontiguousarray(bq, np.float32),
        "bk": np.ascontiguousarray(bk, np.float32),
        "bv": np.ascontiguousarray(bv, np.float32),
        "bo": np.ascontiguousarray(bo, np.float32),
    }
    x = np.ascontiguousarray(x, np.float32)
    in_maps = [dict(shared, x=x[c * BL:(c + 1) * BL]) for c in range(N_CORES)]
    res = run_bass_kernel_spmd(
        nc, in_maps, core_ids=list(range(N_CORES)), trace=trace,
        tmpdir=tmpdir)
    out = np.concatenate([res.results[c]["out"] for c in range(N_CORES)], axis=0)
    attn = np.concatenate([res.results[c]["attn"] for c in range(N_CORES)], axis=0)
    kernel.last_results = res
    return out, attn
